# revision 26
# baseline (speedup 1.0000x reference)
"""GridRNN kernel for Trainium2 (Bass/Tile), 8-core data-parallel over batch.

Structural insight: in this GridRNN, depth-0 inputs are broadcast (x over j,
y over i) and the carry-roll along j is identity on j-constant carries, so by
induction every grid cell depends on only ONE coordinate:
    out[b,d,i,j,0,:] = f_d(b,i)   (hx, independent of j)
    out[b,d,i,j,1,:] = g_d(b,j)   (hy, independent of i)
with tiny 96-step RNN chains:
    f0(i) = tanh(Wx_ih0^T x_i   + Wx_hh0^T f0(i-1) + bx0),  f0(-1)=0
    f1(i) = tanh(Wx_ih1^T f0(i) + Wx_hh1^T f1(i-1) + bx1)
    g0(j) = tanh(Wy_ih0^T y_j   + Wy_hh0^T g0(j-1) + by0)
    g1(j) = tanh(Wy_ih1^T g0((j-1)%96) + Wy_hh1^T g1(j-1) + by1)

Instead of 96 serial (matmul -> tanh) round trips per chain (latency-bound at
~700ns each), each chain is solved parallel-in-time by Jacobi fixed-point
iteration over the whole sequence:
    H <- tanh(C + W_hh^T @ shift(H)),   shift via AP offset into a 97-col tile
Contraction ~0.25/sweep: 9 sweeps reach ~3.9e-3 rel err (bf16 floor ~3.5e-3),
well inside the 2e-2 gate. Each sweep is 2 full-width bf16 matmuls (N=96) +
one fused tanh, so a chain pair costs ~9us instead of ~67us.

Output (18.9MB/core) is assembled in SBUF as [i-partition, (j, hx|hy)] tiles
so every HBM descriptor is a 32KB contiguous run. Tiles live on partitions
0-47 and 64-111: SDMA engine k serves partitions {4k..4k+3, 4k+32..}, so a
0-95 layout loads the even engines 2x while this split loads all 16 evenly;
each store is issued as two 48-partition DMAs on the two HWDGE rings.
The hy half (same data for every i-partition) is replicated across partitions
via a small HBM bounce: write g_d natural once (24KB), read it back with a
stride-0 source AP. d0's read-back rides the idle HWDGE wire before stores
begin; d1's goes through the SWDGE ring so it trickles past d0's store
traffic instead of queuing behind it.
"""

import numpy as np
import ml_dtypes

import concourse.bass as bass
import concourse.bacc as bacc
import concourse.mybir as mybir
import concourse.tile as tile
import concourse.bass_utils as bass_utils

H, S, T, D, B = 128, 96, 96, 2, 8
NITER = 9        # Jacobi sweeps from zero state
QJ = 32          # j-chunk width for output pipelining
NQ = T // QJ
HS = 48          # half of the i range (split-partition layout)
PHI = 64         # partition offset of the upper i half
F32 = mybir.dt.float32
BF16 = mybir.dt.bfloat16
TANH = mybir.ActivationFunctionType.Tanh
BF = ml_dtypes.bfloat16

WNAMES = ["wx_hh0", "wx_ih0", "wy_hh0", "wy_ih0",
          "wx_hh1", "wx_ih1", "wy_hh1", "wy_ih1"]
_off = 0
COLS = {}
for _nm, _w in [("xT", S), ("yT", T), ("ident", H)] + [(n, H) for n in WNAMES]:
    COLS[_nm] = (_off, _off + _w)
    _off += _w
NCOLS = _off

_PROG = None


def _build_program():
    nc = bacc.Bacc("TRN2", target_bir_lowering=False, debug=False)

    cb_h = nc.dram_tensor("consts_bf", [H, NCOLS], BF16, kind="ExternalInput")
    cf_h = nc.dram_tensor("consts_f32", [H, 4], F32, kind="ExternalInput")
    out_h = nc.dram_tensor("out", [D, S, T, 2, H], F32, kind="ExternalOutput")
    scr_h = nc.dram_tensor("scratch", [D, T, H], BF16, kind="Internal")

    with tile.TileContext(nc) as tc:
        with (
            tc.tile_pool(name="const", bufs=1) as cpool,
            tc.tile_pool(name="chains", bufs=1) as chpool,
            tc.tile_pool(name="nat", bufs=1) as natpool,
            tc.tile_pool(name="grep", bufs=2) as gpool,
            tc.tile_pool(name="ot", bufs=3) as otpool,
            tc.tile_pool(name="ps", bufs=4, space="PSUM") as pspool,
            tc.tile_pool(name="pst", bufs=2, space="PSUM") as pstpool,
        ):
            consb = cpool.tile([H, NCOLS], BF16, tag="consb", name="consb")
            consf = cpool.tile([H, 4], F32, tag="consf", name="consf")
            nc.sync.dma_start(consb[:, :], cb_h[:, :])
            nc.sync.dma_start(consf[:, :], cf_h[:, :])

            def sb(nm):
                a, b_ = COLS[nm]
                return consb[:, a:b_]

            # chain state tiles: col 0 is the permanent zero boundary state
            Ht = {c: chpool.tile([H, S + 1], BF16, tag=c, name=c)
                  for c in ["f0", "g0", "f1", "g1"]}
            # natural-layout chains in the split-partition layout:
            # value t lives at partition t (t<48) / t+16 (t>=48)
            nat = {c: natpool.tile([PHI + HS, H], BF16, tag=f"n{c}",
                                   name=f"n{c}")
                   for c in ["f0", "g0", "f1", "g1"]}
            dummy = cpool.tile([H, 1], BF16, tag="dummy", name="dummy")
            for c in ["f0", "g0", "f1", "g1"]:
                nc.vector.memset(Ht[c][:, :], 0.0)
            # pull the tanh table load (~2.7us) off the critical path
            nc.scalar.activation(dummy[:, :], Ht["f0"][:, 0:1], TANH)

            def jacobi_gen(cname, w_hh, w_ih, rhs_in, bias_i):
                Hc = Ht[cname]
                for _ in range(NITER):
                    ps = pspool.tile([H, S], F32, tag="ps", name="ps")
                    nc.tensor.matmul(ps[:, :], sb(w_hh), Hc[:, 0:S],
                                     start=True, stop=False)
                    nc.tensor.matmul(ps[:, :], sb(w_ih), rhs_in,
                                     start=False, stop=True)
                    nc.scalar.activation(Hc[:, 1:S + 1], ps[:, :], TANH,
                                         bias=consf[:, bias_i:bias_i + 1])
                    yield

            def jacobi_pair(specs):
                # interleave two independent chains' sweeps so engines pipeline
                gens = [jacobi_gen(*s) for s in specs]
                while True:
                    done = True
                    for it in gens:
                        try:
                            next(it)
                            done = False
                        except StopIteration:
                            pass
                    if done:
                        break

            def to_natural(cname):
                # two PE transposes land the chain in the split layout
                pst = pstpool.tile([PHI + HS, H], BF16, tag="pst", name="pst")
                nc.tensor.transpose(pst[0:HS, :], Ht[cname][:, 1:HS + 1],
                                    sb("ident"))
                nc.tensor.transpose(pst[PHI:PHI + HS, :],
                                    Ht[cname][:, HS + 1:S + 1], sb("ident"))
                nc.vector.tensor_copy(nat[cname][0:HS, :], pst[0:HS, :])
                nc.vector.tensor_copy(nat[cname][PHI:PHI + HS, :],
                                      pst[PHI:PHI + HS, :])

            def replicate(d, gname, eng):
                # bounce g_d natural through HBM to replicate it onto every
                # used partition; read back in j-quarters so fills can start
                # as soon as their slice lands
                nc2 = nat[gname]
                eng.dma_start(scr_h[d, 0:HS, :], nc2[0:HS, :])
                eng.dma_start(scr_h[d, HS:S, :], nc2[PHI:PHI + HS, :])
                grep_t = gpool.tile([PHI + HS, T * H], BF16, tag="grep",
                                    name=f"grep{d}")
                s = scr_h[d, :, :]
                for q in range(NQ):
                    a, b_ = q * QJ * H, (q + 1) * QJ * H
                    src = bass.AP(s.tensor, s.offset + a,
                                  [[0, PHI + HS], [1, QJ * H]])
                    eng.dma_start(grep_t[:, a:b_], src)
                return grep_t

            def make_ot(d, q):
                return otpool.tile([PHI + HS, QJ * 2 * H], F32, tag="ot",
                                   name=f"ot{d}_{q}")

            def fill_f(ot, fname):
                # hx half: per-partition broadcast of f(i) along j
                fn = nat[fname][:, :]
                src_f = bass.AP(fn.tensor, fn.offset,
                                [fn.ap[0], [0, QJ], fn.ap[1]])
                dst_f = bass.AP(ot.tensor, ot.offset,
                                [ot.ap[0], [2 * H, QJ], [1, H]])
                nc.vector.tensor_copy(dst_f, src_f)

            def fill_g(ot, q, grep_t):
                # hy half: strided expansion of the replicated g row
                gsl = grep_t[:, q * QJ * H:(q + 1) * QJ * H]
                src_g = bass.AP(gsl.tensor, gsl.offset,
                                [gsl.ap[0], [H, QJ], [1, H]])
                dst_g = bass.AP(ot.tensor, ot.offset + H,
                                [ot.ap[0], [2 * H, QJ], [1, H]])
                nc.vector.tensor_copy(dst_g, src_g)

            def store(ot, d, q):
                # two 48-partition stores on separate rings: together they
                # load all 16 SDMA engines evenly
                js = slice(q * QJ, (q + 1) * QJ)
                o_lo = out_h[d, 0:HS, js, :, :]
                o_hi = out_h[d, HS:S, js, :, :]
                lo = ot[0:HS, :]
                hi = ot[PHI:PHI + HS, :]
                src_lo = bass.AP(lo.tensor, lo.offset,
                                 [lo.ap[0], [2 * H, QJ], [H, 2], [1, H]])
                src_hi = bass.AP(hi.tensor, hi.offset,
                                 [hi.ap[0], [2 * H, QJ], [H, 2], [1, H]])
                nc.sync.dma_start(o_lo, src_lo)
                nc.scalar.dma_start(o_hi, src_hi)

            # ---- depth 0 ----
            jacobi_pair([
                ("f0", "wx_hh0", "wx_ih0", sb("xT"), 0),
                ("g0", "wy_hh0", "wy_ih0", sb("yT"), 1),
            ])
            # roll fix: g1's input at j is g0[(j-1)%96]; col 0 := g0[95]
            nc.vector.tensor_copy(Ht["g0"][:, 0:1], Ht["g0"][:, S:S + 1])
            to_natural("f0")
            to_natural("g0")
            grep0 = replicate(0, "g0", nc.sync)   # wire is idle here

            # ---- depth 1 (PE/ACT) runs while depth-0 output streams
            jacobi_pair([
                ("f1", "wx_hh1", "wx_ih1", Ht["f0"][:, 1:S + 1], 2),
                ("g1", "wy_hh1", "wy_ih1", Ht["g0"][:, 0:S], 3),
            ])

            for q in range(NQ):
                ot = make_ot(0, q)
                fill_g(ot, q, grep0)
                fill_f(ot, "f0")
                store(ot, 0, q)

            to_natural("f1")
            to_natural("g1")
            # SWDGE ring: trickles past the HWDGE store traffic
            grep1 = replicate(1, "g1", nc.gpsimd)
            for q in range(NQ):
                ot = make_ot(1, q)
                fill_g(ot, q, grep1)
                fill_f(ot, "f1")
                store(ot, 1, q)

    return nc


def _get_program():
    global _PROG
    if _PROG is None:
        _PROG = _build_program()
        _PROG.finalize()
    return _PROG


TRACE = False
LAST_RESULT = [None]


def kernel(x, y, Wx_ih, Wx_hh, bx_ih, bx_hh, Wy_ih, Wy_hh, by_ih, by_hh,
           batch_size=8, src_len=96, trg_len=96, **_ignored):
    x = np.asarray(x, dtype=np.float32)
    y = np.asarray(y, dtype=np.float32)

    nc = _get_program()

    wparts = {
        "ident": np.eye(H, dtype=BF),
        "wx_hh0": np.asarray(Wx_hh, np.float32)[0].astype(BF),
        "wx_ih0": np.asarray(Wx_ih, np.float32)[0].astype(BF),
        "wy_hh0": np.asarray(Wy_hh, np.float32)[0].astype(BF),
        "wy_ih0": np.asarray(Wy_ih, np.float32)[0].astype(BF),
        "wx_hh1": np.asarray(Wx_hh, np.float32)[1].astype(BF),
        "wx_ih1": np.asarray(Wx_ih, np.float32)[1].astype(BF),
        "wy_hh1": np.asarray(Wy_hh, np.float32)[1].astype(BF),
        "wy_ih1": np.asarray(Wy_ih, np.float32)[1].astype(BF),
    }
    biases = np.stack([
        np.asarray(bx_ih, np.float32)[0] + np.asarray(bx_hh, np.float32)[0],
        np.asarray(by_ih, np.float32)[0] + np.asarray(by_hh, np.float32)[0],
        np.asarray(bx_ih, np.float32)[1] + np.asarray(bx_hh, np.float32)[1],
        np.asarray(by_ih, np.float32)[1] + np.asarray(by_hh, np.float32)[1],
    ], axis=1)  # [H, 4]

    in_maps = []
    for bi in range(B):
        consb = np.empty((H, NCOLS), dtype=BF)
        consb[:, COLS["xT"][0]:COLS["xT"][1]] = x[bi].T.astype(BF)
        consb[:, COLS["yT"][0]:COLS["yT"][1]] = y[bi].T.astype(BF)
        for nm, arr in wparts.items():
            a, b_ = COLS[nm]
            consb[:, a:b_] = arr
        in_maps.append({"consts_bf": consb, "consts_f32": biases})

    res = bass_utils.run_bass_kernel_spmd(
        nc, in_maps, core_ids=list(range(B)), trace=TRACE)
    LAST_RESULT[0] = res
    return np.stack([res.results[c]["out"] for c in range(B)], axis=0)


# revision 27
# speedup vs baseline: 1.3620x; 1.3620x over previous
"""GridRNN kernel for Trainium2 (Bass/Tile), 8-core data-parallel over batch.

Structural insight: in this GridRNN, depth-0 inputs are broadcast (x over j,
y over i) and the carry-roll along j is identity on j-constant carries, so by
induction every grid cell depends on only ONE coordinate:
    out[b,d,i,j,0,:] = f_d(b,i)   (hx, independent of j)
    out[b,d,i,j,1,:] = g_d(b,j)   (hy, independent of i)
with tiny 96-step RNN chains:
    f0(i) = tanh(Wx_ih0^T x_i   + Wx_hh0^T f0(i-1) + bx0),  f0(-1)=0
    f1(i) = tanh(Wx_ih1^T f0(i) + Wx_hh1^T f1(i-1) + bx1)
    g0(j) = tanh(Wy_ih0^T y_j   + Wy_hh0^T g0(j-1) + by0)
    g1(j) = tanh(Wy_ih1^T g0((j-1)%96) + Wy_hh1^T g1(j-1) + by1)

Instead of 96 serial (matmul -> tanh) round trips per chain (latency-bound at
~700ns each), each chain is solved parallel-in-time by Jacobi fixed-point
iteration over the whole sequence:
    H <- tanh(C + W_hh^T @ shift(H)),   shift via AP offset into a 97-col tile
Contraction ~0.25/sweep: 9 sweeps reach ~3.9e-3 rel err (bf16 floor ~3.5e-3),
well inside the 2e-2 gate. Each sweep is 2 full-width bf16 matmuls (N=96) +
one fused tanh, so a chain pair costs ~8us instead of ~67us.

Output (18.9MB/core) is assembled in SBUF as [i-partition, (j, hx|hy)] tiles
so every HBM descriptor is a 32KB contiguous run (separate hx/hy stores would
be 512B runs, which measure ~4x slower). The hy half (same data for every
i-partition) is replicated across partitions via a small HBM bounce: write
g_d natural once (24KB), read it back per j-quarter with a stride-0 source
AP. d0's bounce rides the sync ring before any store is queued; d1's rides
the otherwise-empty scalar ring so it never waits behind store descriptors.
"""

import numpy as np
import ml_dtypes

import concourse.bass as bass
import concourse.bacc as bacc
import concourse.mybir as mybir
import concourse.tile as tile
import concourse.bass_utils as bass_utils

H, S, T, D, B = 128, 96, 96, 2, 8
NITER = 9        # Jacobi sweeps from zero state
QJ = 32          # j-chunk width for output pipelining
NQ = T // QJ
F32 = mybir.dt.float32
BF16 = mybir.dt.bfloat16
TANH = mybir.ActivationFunctionType.Tanh
BF = ml_dtypes.bfloat16

WNAMES = ["wx_hh0", "wx_ih0", "wy_hh0", "wy_ih0",
          "wx_hh1", "wx_ih1", "wy_hh1", "wy_ih1"]
_off = 0
COLS = {}
for _nm, _w in [("xT", S), ("yT", T), ("ident", H)] + [(n, H) for n in WNAMES]:
    COLS[_nm] = (_off, _off + _w)
    _off += _w
NCOLS = _off

_PROG = None


def _build_program():
    nc = bacc.Bacc("TRN2", target_bir_lowering=False, debug=False)

    cb_h = nc.dram_tensor("consts_bf", [H, NCOLS], BF16, kind="ExternalInput")
    cf_h = nc.dram_tensor("consts_f32", [H, 4], F32, kind="ExternalInput")
    out_h = nc.dram_tensor("out", [D, S, T, 2, H], F32, kind="ExternalOutput")
    scr_h = nc.dram_tensor("scratch", [D, T, H], BF16, kind="Internal")

    with tile.TileContext(nc) as tc:
        with (
            tc.tile_pool(name="const", bufs=1) as cpool,
            tc.tile_pool(name="chains", bufs=1) as chpool,
            tc.tile_pool(name="nat", bufs=1) as natpool,
            tc.tile_pool(name="grep", bufs=2) as gpool,
            tc.tile_pool(name="ot", bufs=3) as otpool,
            tc.tile_pool(name="ps", bufs=4, space="PSUM") as pspool,
            tc.tile_pool(name="pst", bufs=2, space="PSUM") as pstpool,
        ):
            consb = cpool.tile([H, NCOLS], BF16, tag="consb", name="consb")
            consf = cpool.tile([H, 4], F32, tag="consf", name="consf")
            nc.sync.dma_start(consb[:, :], cb_h[:, :])
            nc.sync.dma_start(consf[:, :], cf_h[:, :])

            def sb(nm):
                a, b_ = COLS[nm]
                return consb[:, a:b_]

            # chain state tiles: col 0 is the permanent zero boundary state
            Ht = {c: chpool.tile([H, S + 1], BF16, tag=c, name=c)
                  for c in ["f0", "g0", "f1", "g1"]}
            nat = {c: natpool.tile([S, H], BF16, tag=f"n{c}", name=f"n{c}")
                   for c in ["f0", "g0", "f1", "g1"]}
            dummy = cpool.tile([H, 1], BF16, tag="dummy", name="dummy")
            for c in ["f0", "g0", "f1", "g1"]:
                nc.vector.memset(Ht[c][:, :], 0.0)
            # pull the tanh table load (~2.7us) off the critical path
            nc.scalar.activation(dummy[:, :], Ht["f0"][:, 0:1], TANH)

            def jacobi_gen(cname, w_hh, w_ih, rhs_in, bias_i):
                Hc = Ht[cname]
                for _ in range(NITER):
                    ps = pspool.tile([H, S], F32, tag="ps", name="ps")
                    nc.tensor.matmul(ps[:, :], sb(w_hh), Hc[:, 0:S],
                                     start=True, stop=False)
                    nc.tensor.matmul(ps[:, :], sb(w_ih), rhs_in,
                                     start=False, stop=True)
                    nc.scalar.activation(Hc[:, 1:S + 1], ps[:, :], TANH,
                                         bias=consf[:, bias_i:bias_i + 1])
                    yield

            def jacobi_pair(specs):
                # interleave two independent chains' sweeps so engines pipeline
                gens = [jacobi_gen(*s) for s in specs]
                while True:
                    done = True
                    for it in gens:
                        try:
                            next(it)
                            done = False
                        except StopIteration:
                            pass
                    if done:
                        break

            def to_natural(cname):
                pst = pstpool.tile([S, H], BF16, tag="pst", name="pst")
                nc.tensor.transpose(pst[:, :], Ht[cname][:, 1:S + 1], sb("ident"))
                nc.vector.tensor_copy(nat[cname][:, :], pst[:, :])

            def replicate(d, gname, eng):
                # bounce g_d natural through HBM to replicate it onto every
                # partition; read back per j-quarter so fills start as soon
                # as their slice lands
                eng.dma_start(scr_h[d, :, :], nat[gname][:, :])
                grep_t = gpool.tile([S, T * H], BF16, tag="grep", name=f"grep{d}")
                s = scr_h[d, :, :]
                for q in range(NQ):
                    a, b_ = q * QJ * H, (q + 1) * QJ * H
                    src = bass.AP(s.tensor, s.offset + a, [[0, S], [1, QJ * H]])
                    eng.dma_start(grep_t[:, a:b_], src)
                return grep_t

            def make_ot(d, q):
                return otpool.tile([S, QJ * 2 * H], F32, tag="ot",
                                   name=f"ot{d}_{q}")

            def fill_f(ot, fname):
                # hx half: per-partition broadcast of f(i) along j
                fn = nat[fname][:, :]
                src_f = bass.AP(fn.tensor, fn.offset,
                                [fn.ap[0], [0, QJ], fn.ap[1]])
                dst_f = bass.AP(ot.tensor, ot.offset,
                                [ot.ap[0], [2 * H, QJ], [1, H]])
                nc.vector.tensor_copy(dst_f, src_f)

            def fill_g(ot, q, grep_t):
                # hy half: strided expansion of the replicated g row
                gsl = grep_t[:, q * QJ * H:(q + 1) * QJ * H]
                src_g = bass.AP(gsl.tensor, gsl.offset,
                                [gsl.ap[0], [H, QJ], [1, H]])
                dst_g = bass.AP(ot.tensor, ot.offset + H,
                                [ot.ap[0], [2 * H, QJ], [1, H]])
                nc.vector.tensor_copy(dst_g, src_g)

            def store(ot, d, q):
                o = out_h[d, :, q * QJ:(q + 1) * QJ, :, :]
                src_o = bass.AP(ot.tensor, ot.offset,
                                [ot.ap[0], [2 * H, QJ], [H, 2], [1, H]])
                nc.sync.dma_start(o, src_o)

            # ---- depth 0 ----
            jacobi_pair([
                ("f0", "wx_hh0", "wx_ih0", sb("xT"), 0),
                ("g0", "wy_hh0", "wy_ih0", sb("yT"), 1),
            ])
            # roll fix: g1's input at j is g0[(j-1)%96]; col 0 := g0[95]
            nc.vector.tensor_copy(Ht["g0"][:, 0:1], Ht["g0"][:, S:S + 1])
            to_natural("f0")
            to_natural("g0")
            grep0 = replicate(0, "g0", nc.sync)   # sync ring is idle here

            # ---- depth 1 (PE/ACT) runs while depth-0 output streams
            jacobi_pair([
                ("f1", "wx_hh1", "wx_ih1", Ht["f0"][:, 1:S + 1], 2),
                ("g1", "wy_hh1", "wy_ih1", Ht["g0"][:, 0:S], 3),
            ])

            for q in range(NQ):
                ot = make_ot(0, q)
                fill_g(ot, q, grep0)
                fill_f(ot, "f0")
                store(ot, 0, q)

            to_natural("f1")
            to_natural("g1")
            # scalar ring carries only this bounce: never queues behind stores
            grep1 = replicate(1, "g1", nc.scalar)
            for q in range(NQ):
                ot = make_ot(1, q)
                fill_g(ot, q, grep1)
                fill_f(ot, "f1")
                store(ot, 1, q)

    return nc


def _get_program():
    global _PROG
    if _PROG is None:
        _PROG = _build_program()
        _PROG.finalize()
    return _PROG


TRACE = False
LAST_RESULT = [None]


def kernel(x, y, Wx_ih, Wx_hh, bx_ih, bx_hh, Wy_ih, Wy_hh, by_ih, by_hh,
           batch_size=8, src_len=96, trg_len=96, **_ignored):
    x = np.asarray(x, dtype=np.float32)
    y = np.asarray(y, dtype=np.float32)

    nc = _get_program()

    wparts = {
        "ident": np.eye(H, dtype=BF),
        "wx_hh0": np.asarray(Wx_hh, np.float32)[0].astype(BF),
        "wx_ih0": np.asarray(Wx_ih, np.float32)[0].astype(BF),
        "wy_hh0": np.asarray(Wy_hh, np.float32)[0].astype(BF),
        "wy_ih0": np.asarray(Wy_ih, np.float32)[0].astype(BF),
        "wx_hh1": np.asarray(Wx_hh, np.float32)[1].astype(BF),
        "wx_ih1": np.asarray(Wx_ih, np.float32)[1].astype(BF),
        "wy_hh1": np.asarray(Wy_hh, np.float32)[1].astype(BF),
        "wy_ih1": np.asarray(Wy_ih, np.float32)[1].astype(BF),
    }
    biases = np.stack([
        np.asarray(bx_ih, np.float32)[0] + np.asarray(bx_hh, np.float32)[0],
        np.asarray(by_ih, np.float32)[0] + np.asarray(by_hh, np.float32)[0],
        np.asarray(bx_ih, np.float32)[1] + np.asarray(bx_hh, np.float32)[1],
        np.asarray(by_ih, np.float32)[1] + np.asarray(by_hh, np.float32)[1],
    ], axis=1)  # [H, 4]

    in_maps = []
    for bi in range(B):
        consb = np.empty((H, NCOLS), dtype=BF)
        consb[:, COLS["xT"][0]:COLS["xT"][1]] = x[bi].T.astype(BF)
        consb[:, COLS["yT"][0]:COLS["yT"][1]] = y[bi].T.astype(BF)
        for nm, arr in wparts.items():
            a, b_ = COLS[nm]
            consb[:, a:b_] = arr
        in_maps.append({"consts_bf": consb, "consts_f32": biases})

    res = bass_utils.run_bass_kernel_spmd(
        nc, in_maps, core_ids=list(range(B)), trace=TRACE)
    LAST_RESULT[0] = res
    return np.stack([res.results[c]["out"] for c in range(B)], axis=0)
